# revision 13
# baseline (speedup 1.0000x reference)
"""BertAdapter (TT-decomposed bottleneck MLP) Trainium2 kernel.

Computes  out = x + gelu(x @ W_down + b_down) @ W_up + b_up  where the
adapter weights arrive as tensor-train cores.  The TT cores are tiny
(~50K params), so they are contracted to dense matrices on the host and
the device kernel runs the dense bottleneck MLP data-parallel across
8 NeuronCores (2 batches of 2048 tokens per core).

Per-core device kernel, per 512-token block:
  1. DMA in four [128, 768] x tiles (tokens on partitions).
  2. PE-transpose them to xT [768-chunked, 512] (fp32, via identity).
  3. Down-proj: 6 accumulating matmuls  W_down_chunk.T @ xT_chunk ->
     PSUM [64, 512]  (weights stationary, contraction over hidden).
  4. Exact-erf Gelu + b_down bias on the scalar engine (bias is
     per-partition in this transposed layout).
  5. Up-proj: actT serves directly as the stationary operand, so the
     output lands back in natural [token, hidden] layout; b_up is folded
     in via an appended ones-row on actT / b_up row on W_up.
  6. Residual add (x + up) on the vector engine, DMA out.

Matmuls run in float32r (TF32) mode: full-rate on the PE array vs 1/4
rate for strict fp32, with ~10-bit mantissa rounding only inside the
adapter branch (the residual path stays exact fp32).
"""

import os
import sys
from contextlib import ExitStack

import numpy as np

for _p in ("/opt/trn_rl_repo", "/root/.axon_site/_ro/trn_rl_repo"):
    if os.path.isdir(_p) and _p not in sys.path:
        sys.path.insert(0, _p)

import concourse.bass as bass
import concourse.tile as tile
from concourse import mybir
from concourse.bass_utils import run_bass_kernel_spmd
from concourse.masks import make_identity
from concourse.vector_clock import ScopedClock

P = 128                 # SBUF partitions
H = 768                 # hidden size
A = 64                  # adapter bottleneck size
B, S = 16, 2048         # full batch / seq
NCORES = 8
TOK = (B // NCORES) * S  # tokens per core = 4096
TBLK = 512              # tokens per pipeline block
NBLK = TOK // TBLK
HC = H // P             # hidden chunks of 128
TSUB = TBLK // P        # 128-token subtiles per block
F32 = mybir.dt.float32
USE_F32R = True         # TF32 matmul mode (full-rate fp32 storage)


_TileContext = tile.TileContext


def _legalize_waits(nc):
    """Split multi-wait instructions for this walrus build.

    The walrus in this toolchain accepts only ONE sync-wait per
    instruction ("Too many sync wait commands" in setupSyncWait), while
    Tile freely attaches several.  Hoist all but the last wait of each
    instruction onto freshly inserted same-engine NoOps directly before
    it — engine program order makes this semantically identical.
    """
    n = 0

    def fix_block(bb):
        nonlocal n
        insts = bb.instructions
        i = 0
        while i < len(insts):
            inst = insts[i]
            for sub in getattr(inst, "blocks", None) or []:
                fix_block(sub)
            si = inst.sync_info
            waits = list(si.on_wait) if si and si.on_wait else []
            if len(waits) > 1:
                for w in waits[:-1]:
                    nop = mybir.InstNoOp(name=f"I-waitsplit-{n}", ins=[], outs=[])
                    n += 1
                    nop.engine = inst.engine
                    nop.sync_info = mybir.SyncInfo(on_wait=[w], on_update=[])
                    insts.insert(i, nop)
                    i += 1
                inst.sync_info = mybir.SyncInfo(
                    on_wait=[waits[-1]], on_update=list(si.on_update)
                )
            i += 1

    for fn in nc.m.functions:
        for bb in fn.blocks:
            fix_block(bb)
    return nc


MMDT = mybir.dt.float32r if USE_F32R else F32


def build_nc(tok=TOK, repeats=1):
    nblk = tok // TBLK
    nc = bass.Bass("TRN2", target_bir_lowering=False, debug=False)
    x = nc.dram_tensor("x", [tok, H], F32, kind="ExternalInput").ap()
    # wd/bd carry an extra adapter column: wd col A is zeros and bd[A] is
    # gelu^-1(1.0), so the gelu writes a constant ones-row into act[A] that
    # multiplies the b_up row of wub in the up-projection (bias via matmul).
    wd = nc.dram_tensor("wd", [H, A + 1], F32, kind="ExternalInput").ap()
    wub = nc.dram_tensor("wub", [A + 1, H], F32, kind="ExternalInput").ap()
    bd = nc.dram_tensor("bd", [A + 1, 1], F32, kind="ExternalInput").ap()
    y = nc.dram_tensor("y", [tok, H], F32, kind="ExternalOutput").ap()

    with ExitStack() as ctx:
        tc = ctx.enter_context(_TileContext(nc))
        const = ctx.enter_context(tc.tile_pool(name="const", bufs=1))
        xin = ctx.enter_context(tc.tile_pool(name="xin", bufs=10))
        xtp = ctx.enter_context(tc.tile_pool(name="xt", bufs=2))
        actp = ctx.enter_context(tc.tile_pool(name="act", bufs=2))
        outp = ctx.enter_context(tc.tile_pool(name="out", bufs=4))
        ps_t = ctx.enter_context(tc.tile_pool(name="ps_t", bufs=2, space="PSUM"))
        ps_d = ctx.enter_context(tc.tile_pool(name="ps_d", bufs=2, space="PSUM"))
        ps_u = ctx.enter_context(tc.tile_pool(name="ps_u", bufs=2, space="PSUM"))

        ident = const.tile([P, P], F32)
        make_identity(nc, ident)
        # weights: DMA to fp32 staging, engine-copy to MMDT (the fp32r
        # matmul verifier requires inputs produced by a rounding engine op)
        wd_st = const.tile([P, HC, A + 1], F32)
        nc.sync.dma_start(wd_st[:], wd.rearrange("(c p) a -> p c a", p=P))
        wub_st = const.tile([A + 1, H], F32)
        nc.sync.dma_start(wub_st[:], wub[:])
        wd_sb = const.tile([P, HC, A + 1], MMDT)
        wub_sb = const.tile([A + 1, H], MMDT)
        if USE_F32R:
            nc.vector.tensor_copy(wd_sb[:], wd_st[:])
            nc.vector.tensor_copy(wub_sb[:], wub_st[:])
        else:
            wd_sb, wub_sb = wd_st, wub_st
        bd_sb = const.tile([A + 1, 1], F32)
        nc.sync.dma_start(bd_sb[:], bd[:])

        for b in range(nblk * repeats):
            b = b % nblk
            t0 = b * TBLK
            xs = []
            for i in range(TSUB):
                xt_in = xin.tile([P, H], F32, tag="xin")
                nc.sync.dma_start(xt_in[:], x[t0 + i * P : t0 + (i + 1) * P, :])
                xs.append(xt_in)
            # transpose x -> xT (hidden on partitions)
            xt_sb = xtp.tile([P, HC, TBLK], MMDT)
            for j in range(HC):
                pt = ps_t.tile([P, TBLK], F32)
                for i in range(TSUB):
                    nc.tensor.transpose(
                        pt[:, i * P : (i + 1) * P],
                        xs[i][:, j * P : (j + 1) * P],
                        ident[:],
                    )
                if j % 3 == 2:
                    nc.vector.tensor_copy(xt_sb[:, j, :], pt[:])
                else:
                    nc.scalar.copy(xt_sb[:, j, :], pt[:])
            # down projection: accumulate over hidden chunks
            pd = ps_d.tile([A + 1, TBLK], F32)
            for j in range(HC):
                nc.tensor.matmul(
                    pd[:],
                    wd_sb[:, j, :],
                    xt_sb[:, j, :],
                    start=(j == 0),
                    stop=(j == HC - 1),
                )
            # exact-erf gelu with per-partition b_down bias; row A computes
            # gelu(0 + gelu^-1(1)) = 1.0, the b_up multiplier
            act = actp.tile([A + 1, TBLK], MMDT)
            nc.scalar.activation(
                act[:], pd[:], mybir.ActivationFunctionType.Gelu,
                bias=bd_sb[:, 0:1],
            )
            # up projection back to natural [token, hidden] layout + residual
            for i in range(TSUB):
                pu = ps_u.tile([P, H], F32)
                lhsT = act[:, i * P : (i + 1) * P]
                nc.tensor.matmul(
                    pu[:, 0:512], lhsT, wub_sb[:, 0:512],
                    start=True, stop=True,
                )
                nc.tensor.matmul(
                    pu[:, 512:H], lhsT, wub_sb[:, 512:H],
                    start=True, stop=True,
                )
                ot = outp.tile([P, H], F32)
                nc.vector.tensor_add(ot[:], xs[i][:], pu[:])
                nc.sync.dma_start(y[t0 + i * P : t0 + (i + 1) * P, :], ot[:])
    return _legalize_waits(nc)


def _tt_to_matrix(cores, in_dim, out_dim):
    t = cores[0]
    for c in cores[1:]:
        t = np.tensordot(t, c, axes=([-1], [0]))
    t = np.squeeze(t, axis=(0, -1))
    return np.ascontiguousarray(t.reshape(in_dim, out_dim).astype(np.float32))


def _gelu_inv_one():
    """x with x * Phi(x) == 1 (erf gelu), solved by Newton in float64."""
    import math

    def gelu(x):
        return x * 0.5 * (1.0 + math.erf(x / math.sqrt(2.0)))

    def dgelu(x):
        return 0.5 * (1.0 + math.erf(x / math.sqrt(2.0))) + x * math.exp(
            -0.5 * x * x
        ) / math.sqrt(2.0 * math.pi)

    x = 1.15
    for _ in range(40):
        x -= (gelu(x) - 1.0) / dgelu(x)
    return x


_NC_CACHE = {}


def _get_nc(tok=TOK):
    if tok not in _NC_CACHE:
        _NC_CACHE[tok] = build_nc(tok)
    return _NC_CACHE[tok]


def kernel(hidden_states, d0, d1, d2, d3, d4, u0, u1, u2, u3, u4,
           b_down, b_up, **_run_kwargs):
    hs = np.asarray(hidden_states, dtype=np.float32)
    w_down = _tt_to_matrix(
        [np.asarray(c, np.float32) for c in (d0, d1, d2, d3, d4)], H, A
    )
    w_up = _tt_to_matrix(
        [np.asarray(c, np.float32) for c in (u0, u1, u2, u3, u4)], A, H
    )
    wd = np.concatenate([w_down, np.zeros((H, 1), np.float32)], axis=1)
    wd = np.ascontiguousarray(wd)
    wub = np.ascontiguousarray(
        np.concatenate([w_up, np.asarray(b_up, np.float32)[None, :]], axis=0)
    )
    bd = np.concatenate(
        [
            np.asarray(b_down, np.float32).reshape(A, 1),
            np.full((1, 1), _gelu_inv_one(), np.float32),
        ],
        axis=0,
    )
    bd = np.ascontiguousarray(bd)

    bpc = B // NCORES
    flat = hs.reshape(B * S, H)
    in_maps = [
        {
            "x": np.ascontiguousarray(flat[c * TOK : (c + 1) * TOK]),
            "wd": wd,
            "wub": wub,
            "bd": bd,
        }
        for c in range(NCORES)
    ]
    nc = _get_nc()
    res = run_bass_kernel_spmd(nc, in_maps, list(range(NCORES)), **_run_kwargs)
    out = np.concatenate([res.results[c]["y"] for c in range(NCORES)], axis=0)
    out = out.reshape(B, S, H)
    if _run_kwargs:
        kernel.last_results = res
    return out
